# revision 4
# baseline (speedup 1.0000x reference)
"""Trainium2 Bass kernel for ConvolutionalSelfAttention.

Problem (hardcoded): x [8, 64, 64, 64] f32 (B, C, H, W), Wq/Wk/Wv [64, 64],
bq/bk/bv [64]. 7x7 'valid' windows (58x58 per image), query = window center
pixel, keys/values = whole window, softmax(q.k/sqrt(C)) @ v.
Output [8, 58, 58, 64] f32.

Strategy: data-parallel over batch, one image per NeuronCore (8 cores).
Per core:
  - QKV projected once per PIXEL (not per window-membership: 40x less work):
      q, k in channel-major layout [C=64p, H, W]   (PE, fp32r full-rate)
      v  in spatial-major layout  [HW=32x128p, 65] (65th col = ones, gives
                                                    the softmax denominator)
  - Windowed attention as banded matmuls over row-bands: for each pair of
    window rows i, i+1 the keys span image rows i..i+8 = 512 pixels = 4
    chunks of 128. scores_T[k, j] via 4 matmuls (lhsT = k-chunk [64,128],
    rhs = q2 [64,116]); exp on ACT; band mask applied multiplicatively
    (fp16); out[j, c] + denominator via 4 PSUM-accumulated matmuls with
    lhsT = attn chunk [128,116], rhs = v chunk [128,65]; normalize with
    per-partition reciprocal on DVE. Biases fold in via a ones-row on x
    and bias-rows on the weights.
"""

import numpy as np

B, C, H, W, K = 8, 64, 64, 64, 7
HC = WC = H - K + 1          # 58
N = HC * WC                  # 3364
NPAIR = HC // 2              # 29 window-row pairs
JW = 2 * WC                  # 116 windows per pair
SCALE = float(1.0 / np.sqrt(C))

_CACHE = {}


def _build_mask_np():
    """[128, 4, 116] multiplicative band mask, shared by every row-pair."""
    kk = np.arange(128)[:, None, None]
    c = np.arange(4)[None, :, None]
    col = np.arange(JW)[None, None, :]
    k_local = c * 128 + kk            # [0, 512) position within the row band
    dI, jp = k_local // W, k_local % W
    jb, j = col // WC, col % WC
    ok = (dI - jb >= 0) & (dI - jb < K) & (jp - j >= 0) & (jp - j < K)
    return ok.astype(np.float16)


def _build_module(attn_dt_name="float16"):
    import concourse.tile as tile
    from concourse import bacc, mybir

    dt = mybir.dt
    f32 = dt.float32
    f32r = dt.float32r
    f16 = getattr(dt, attn_dt_name)

    nc = bacc.Bacc(
        "TRN2", target_bir_lowering=False, debug=False, enable_asserts=False,
        num_devices=8,
    )

    x_d = nc.dram_tensor("x65", [65, H, W], f32r, kind="ExternalInput").ap()
    wq_d = nc.dram_tensor("wqt", [65, C], f32r, kind="ExternalInput").ap()
    wk_d = nc.dram_tensor("wkt", [65, C], f32r, kind="ExternalInput").ap()
    wv_d = nc.dram_tensor("wvt", [65, C], f32r, kind="ExternalInput").ap()
    mask_d = nc.dram_tensor("mask", [128, 4, JW], f16, kind="ExternalInput").ap()
    out_d = nc.dram_tensor("out", [N, C], f32, kind="ExternalOutput").ap()

    with tile.TileContext(nc) as tc:
        with (
            tc.tile_pool(name="const", bufs=1) as const,
            tc.tile_pool(name="qk", bufs=1) as qkpool,
            tc.tile_pool(name="attn", bufs=3) as attnpool,
            tc.tile_pool(name="fin", bufs=3) as finpool,
        ):
            x_sb = const.tile([65, H, W], f32r)
            wq_sb = const.tile([65, C], f32r)
            wk_sb = const.tile([65, C], f32r)
            wv_sb = const.tile([65, C], f32r)
            mask_sb = const.tile([128, 4, JW], f16)
            q_sb = qkpool.tile([C, H, W], f16, tag="q")
            k_sb = qkpool.tile([C, H, W], f16, tag="k")
            v_sb = qkpool.tile([128, 32, C + 1], f16, tag="v")

            nc.sync.dma_start(x_sb[:], x_d[:])
            nc.sync.dma_start(wq_sb[:], wq_d[:])
            nc.sync.dma_start(wk_sb[:], wk_d[:])
            nc.sync.dma_start(wv_sb[:], wv_d[:])
            nc.sync.dma_start(mask_sb[:], mask_d[:])
            nc.gpsimd.memset(v_sb[:], 1.0)  # ones column (col 64 of each chunk)

            # ---- QKV projections ----
            with (
                tc.tile_pool(name="psqk", bufs=3, space="PSUM") as psqk,
                tc.tile_pool(name="psv", bufs=3, space="PSUM") as psv,
            ):
                # q, k channel-major: out[c_out, pix] = sum_cin W[cin, cout] x[cin, pix]
                for s in range(8):
                    ps = psqk.tile([C, 8, W], f32, tag="ps")
                    nc.tensor.matmul(
                        ps[:],
                        wq_sb[:],
                        x_sb[:, s * 8:(s + 1) * 8, :],
                    )
                    nc.scalar.copy(q_sb[:, s * 8:(s + 1) * 8, :], ps[:])
                for s in range(8):
                    ps = psqk.tile([C, 8, W], f32, tag="ps")
                    nc.tensor.matmul(
                        ps[:],
                        wk_sb[:],
                        x_sb[:, s * 8:(s + 1) * 8, :],
                    )
                    nc.vector.tensor_copy(k_sb[:, s * 8:(s + 1) * 8, :], ps[:])
                # v spatial-major: out[pix, c_out] = sum_cin x[cin, pix] W[cin, cout]
                for r in range(32):
                    ps = psv.tile([128, C], f32, tag="psv")
                    nc.tensor.matmul(
                        ps[:],
                        x_sb[:, 2 * r:2 * r + 2, :],
                        wv_sb[:],
                    )
                    if r % 2 == 0:
                        nc.scalar.copy(v_sb[:, r, 0:C], ps[:])
                    else:
                        nc.vector.tensor_copy(v_sb[:, r, 0:C], ps[:])

            # ---- banded attention over window-row pairs ----
            with (
                tc.tile_pool(name="pssc", bufs=2, space="PSUM") as pssc,
                tc.tile_pool(name="psout", bufs=2, space="PSUM") as psout,
            ):
                scores = [None] * NPAIR

                def emit_scores(p):
                    i = 2 * p
                    sc = pssc.tile([128, 4, JW], f32, tag="sc")
                    q2 = q_sb[:, i + 3:i + 5, 3:3 + WC]     # [64, 2, 58]
                    for c in range(4):
                        nc.tensor.matmul(
                            sc[:, c, :],
                            k_sb[:, i + 2 * c:i + 2 * c + 2, :],  # [64, 2, 64]
                            q2,
                        )
                    scores[p] = sc

                def emit_tail(p):
                    sc = scores[p]
                    ex = attnpool.tile([128, 4, JW], f16, tag="ex")
                    nc.scalar.activation(
                        ex[:], sc[:], mybir.ActivationFunctionType.Exp,
                        scale=SCALE,
                    )
                    at = attnpool.tile([128, 4, JW], f16, tag="at")
                    nc.vector.tensor_mul(at[:], ex[:], mask_sb[:])
                    ops = psout.tile([JW, C + 1], f32, tag="ops")
                    for c in range(4):
                        nc.tensor.matmul(
                            ops[:],
                            at[:, c, :],            # [128, 116]
                            v_sb[:, p + c, :],      # [128, 65]
                            start=(c == 0), stop=(c == 3),
                        )
                    recip = finpool.tile([JW, 1], f32, tag="recip")
                    nc.vector.reciprocal(recip[:], ops[:, C:C + 1])
                    outt = finpool.tile([JW, C], f32, tag="outt")
                    nc.vector.tensor_scalar(
                        outt[:], ops[:, 0:C], recip[:], None,
                        mybir.AluOpType.mult,
                    )
                    nc.sync.dma_start(out_d[p * JW:(p + 1) * JW, :], outt[:])

                for p in range(NPAIR):
                    emit_scores(p)
                    if p >= 1:
                        emit_tail(p - 1)
                emit_tail(NPAIR - 1)

    nc.compile()
    return nc


def _get_module():
    if "nc" not in _CACHE:
        _CACHE["nc"] = _build_module()
        _CACHE["mask"] = _build_mask_np()
    return _CACHE["nc"], _CACHE["mask"]


def _make_in_maps(x, Wq, bq, Wk, bk, Wv, bv, mask):
    wqt = np.concatenate([Wq, bq[None]]).astype(np.float32)
    wkt = np.concatenate([Wk, bk[None]]).astype(np.float32)
    wvt = np.concatenate([Wv, bv[None]]).astype(np.float32)
    ones = np.ones((1, H, W), np.float32)
    in_maps = []
    for b in range(B):
        x65 = np.concatenate([np.asarray(x[b], np.float32), ones])
        in_maps.append({
            "x65": np.ascontiguousarray(x65),
            "wqt": wqt, "wkt": wkt, "wvt": wvt,
            "mask": mask,
        })
    return in_maps


def run(inputs, trace=False, **spmd_kwargs):
    """Returns (output [8, 58, 58, 64] f32, BassKernelResults)."""
    from concourse import bass_utils

    nc, mask = _get_module()
    in_maps = _make_in_maps(
        inputs["x"], inputs["Wq"], inputs["bq"], inputs["Wk"], inputs["bk"],
        inputs["Wv"], inputs["bv"], mask,
    )
    res = bass_utils.run_bass_kernel_spmd(
        nc, in_maps, core_ids=list(range(B)), trace=trace, **spmd_kwargs,
    )
    out = np.stack(
        [res.results[b]["out"].reshape(HC, WC, C) for b in range(B)]
    ).astype(np.float32)
    return out, res


def kernel(**inputs) -> np.ndarray:
    return run(inputs)[0]


# revision 7
# speedup vs baseline: 1.0641x; 1.0641x over previous
"""Trainium2 Bass kernel for ConvolutionalSelfAttention.

Problem (hardcoded): x [8, 64, 64, 64] f32 (B, C, H, W), Wq/Wk/Wv [64, 64],
bq/bk/bv [64]. 7x7 'valid' windows (58x58 per image), query = window center
pixel, keys/values = whole window, softmax(q.k/sqrt(C)) @ v.
Output [8, 58, 58, 64] f32.

Strategy: data-parallel over batch, one image per NeuronCore (8 cores).
Per core:
  - inputs shipped fp16 (x + ones row, weights + bias rows)
  - QKV projected once per PIXEL: q, k channel-major [C=64p, H, W];
    v spatial-major [32x128p, 65] (65th col = ones -> softmax denominator)
  - windowed attention over window-row pairs (i, i+1), padded to 128
    columns (full image rows as rhs) so every stationary operand has
    exactly 128 weight columns -> fast weight load. Column col of a pair
    maps to window (i + col//64, col%64 - 3); the 12 out-of-range columns
    per pair are masked to a single arbitrary key so their denominators
    stay finite, and never stored.
"""

import numpy as np

B, C, H, W, K = 8, 64, 64, 64, 7
HC = WC = H - K + 1          # 58
N = HC * WC                  # 3364
NPAIR = HC // 2              # 29 window-row pairs
JW = 2 * WC                  # 116 windows per pair
SCALE = float(1.0 / np.sqrt(C))
SCALAR_DMA = False           # issue half the DMAs from the ACT HWDGE ring

_CACHE = {}


def _build_mask_np():
    """[128, 4, 128] multiplicative band mask in the padded-column layout."""
    kk = np.arange(128)[:, None, None]
    c = np.arange(4)[None, :, None]
    col = np.arange(128)[None, None, :]
    k_local = c * 128 + kk            # [0, 512) position within the row band
    dI, jp = k_local // W, k_local % W
    jb, j = col // 64, col % 64 - 3
    ok = (j >= 0) & (j < WC) & (dI - jb >= 0) & (dI - jb < K) \
        & (jp - j >= 0) & (jp - j < K)
    m = ok.astype(np.float16)
    # pad columns (j out of range): one arbitrary key so denom stays finite
    m[0, 0, (np.arange(128) % 64 - 3 < 0) | (np.arange(128) % 64 - 3 >= WC)] = 1.0
    return m


def _build_module():
    import concourse.tile as tile
    from concourse import bacc, mybir

    dt = mybir.dt
    f32 = dt.float32
    f16 = dt.float16

    nc = bacc.Bacc(
        "TRN2", target_bir_lowering=False, debug=False, enable_asserts=False,
        num_devices=8,
    )

    x_d = nc.dram_tensor("x65", [65, H, W], f16, kind="ExternalInput").ap()
    wq_d = nc.dram_tensor("wqt", [65, C], f16, kind="ExternalInput").ap()
    wk_d = nc.dram_tensor("wkt", [65, C], f16, kind="ExternalInput").ap()
    wv_d = nc.dram_tensor("wvt", [65, C], f16, kind="ExternalInput").ap()
    mask_d = nc.dram_tensor("mask", [128, 4, 128], f16, kind="ExternalInput").ap()
    out_d = nc.dram_tensor("out", [N, C], f32, kind="ExternalOutput").ap()

    with tile.TileContext(nc) as tc:
        with (
            tc.tile_pool(name="const", bufs=1) as const,
            tc.tile_pool(name="qk", bufs=1) as qkpool,
            tc.tile_pool(name="attn", bufs=4) as attnpool,
            tc.tile_pool(name="fin", bufs=4) as finpool,
        ):
            x_sb = const.tile([65, H, W], f16)
            wq_sb = const.tile([65, C], f16)
            wk_sb = const.tile([65, C], f16)
            wv_sb = const.tile([65, C], f16)
            mask_sb = const.tile([128, 4, 128], f16)
            q_sb = qkpool.tile([C, H, W], f16, tag="q")
            k_sb = qkpool.tile([C, H, W], f16, tag="k")
            v_sb = qkpool.tile([128, 32, C + 1], f16, tag="v")

            eng2 = nc.scalar if SCALAR_DMA else nc.sync
            eng2.dma_start(wq_sb[:], wq_d[:])
            eng2.dma_start(wk_sb[:], wk_d[:])
            eng2.dma_start(wv_sb[:], wv_d[:])
            nc.sync.dma_start(x_sb[:, 0:32, :], x_d[:, 0:32, :])
            eng2.dma_start(x_sb[:, 32:64, :], x_d[:, 32:64, :])
            nc.sync.dma_start(mask_sb[:], mask_d[:])
            nc.gpsimd.memset(v_sb[:], 1.0)  # ones column (col 64 of each chunk)

            # ---- QKV projections ----
            with (
                tc.tile_pool(name="psqk", bufs=2, space="PSUM") as psqk,
                tc.tile_pool(name="psv", bufs=2, space="PSUM") as psv,
            ):
                # q, k channel-major: out[c_out, pix] = sum_cin W[cin, cout] x[cin, pix]
                # 2 matmuls (N=512 each) per 2-bank PSUM tile, 1 merged copy.
                for g in range(4):
                    ps = psqk.tile([C, 16, W], f32, tag="ps")
                    for h in range(2):
                        s = 2 * g + h
                        nc.tensor.matmul(
                            ps[:, 8 * h:8 * h + 8, :],
                            wq_sb[:],
                            x_sb[:, s * 8:(s + 1) * 8, :],
                        )
                    eng = nc.scalar.copy if g % 2 == 0 else nc.vector.tensor_copy
                    eng(q_sb[:, g * 16:(g + 1) * 16, :], ps[:])
                for g in range(4):
                    ps = psqk.tile([C, 16, W], f32, tag="ps")
                    for h in range(2):
                        s = 2 * g + h
                        nc.tensor.matmul(
                            ps[:, 8 * h:8 * h + 8, :],
                            wk_sb[:],
                            x_sb[:, s * 8:(s + 1) * 8, :],
                        )
                    eng = nc.scalar.copy if g % 2 == 1 else nc.vector.tensor_copy
                    eng(k_sb[:, g * 16:(g + 1) * 16, :], ps[:])
                # v spatial-major: out[pix, c_out] = sum_cin x[cin, pix] W[cin, cout]
                for g in range(8):
                    ps = psv.tile([128, 4, C], f32, tag="psv")
                    for h in range(4):
                        r = 4 * g + h
                        nc.tensor.matmul(
                            ps[:, h, :],
                            x_sb[:, 2 * r:2 * r + 2, :],
                            wv_sb[:],
                        )
                    eng = nc.scalar.copy if g % 2 == 0 else nc.vector.tensor_copy
                    eng(v_sb[:, 4 * g:4 * g + 4, 0:C], ps[:])

            # ---- banded attention over window-row pairs (padded-128 layout) ----
            with (
                tc.tile_pool(name="pssc", bufs=3, space="PSUM") as pssc,
                tc.tile_pool(name="psout", bufs=3, space="PSUM") as psout,
            ):
                scores = [None] * NPAIR

                def emit_scores(p):
                    i = 2 * p
                    sc = pssc.tile([128, 4, 128], f32, tag="sc")
                    q2 = q_sb[:, i + 3:i + 5, :]            # [64, 2, 64] = N=128
                    for c in range(4):
                        nc.tensor.matmul(
                            sc[:, c, :],
                            k_sb[:, i + 2 * c:i + 2 * c + 2, :],  # [64, 2, 64]
                            q2,
                        )
                    scores[p] = sc

                def emit_tail(p):
                    sc = scores[p]
                    ex = attnpool.tile([128, 4, 128], f16, tag="ex")
                    nc.scalar.activation(
                        ex[:], sc[:], mybir.ActivationFunctionType.Exp,
                        scale=SCALE,
                    )
                    at = attnpool.tile([128, 4, 128], f16, tag="at")
                    nc.vector.tensor_mul(at[:], ex[:], mask_sb[:])
                    ops = psout.tile([128, C + 1], f32, tag="ops")
                    for c in range(4):
                        nc.tensor.matmul(
                            ops[:],
                            at[:, c, :],            # [128, 128] -> FWL
                            v_sb[:, p + c, :],      # [128, 65]
                            start=(c == 0), stop=(c == 3),
                        )
                    recip = finpool.tile([128, 1], f32, tag="recip")
                    nc.vector.reciprocal(recip[:], ops[:, C:C + 1])
                    outt = finpool.tile([128, C], f32, tag="outt")
                    if p % 2 == 0:
                        nc.scalar.mul(outt[:], ops[:, 0:C], recip[:])
                    else:
                        nc.vector.tensor_scalar(
                            outt[:], ops[:, 0:C], recip[:], None,
                            mybir.AluOpType.mult,
                        )
                    # rows jb*64 + 3..61 hold windows (2p+jb, 0..58)
                    for jb in range(2):
                        eng = nc.sync if (p + jb) % 2 == 0 else eng2
                        eng.dma_start(
                            out_d[p * JW + jb * WC: p * JW + (jb + 1) * WC, :],
                            outt[jb * 64 + 3: jb * 64 + 3 + WC, :],
                        )

                LAG = 2
                for p in range(NPAIR):
                    emit_scores(p)
                    if p >= LAG:
                        emit_tail(p - LAG)
                for p in range(NPAIR - LAG, NPAIR):
                    emit_tail(p)

    nc.compile()
    return nc


def _get_module():
    if "nc" not in _CACHE:
        _CACHE["nc"] = _build_module()
        _CACHE["mask"] = _build_mask_np()
    return _CACHE["nc"], _CACHE["mask"]


def _make_in_maps(x, Wq, bq, Wk, bk, Wv, bv, mask):
    wqt = np.concatenate([Wq, bq[None]]).astype(np.float16)
    wkt = np.concatenate([Wk, bk[None]]).astype(np.float16)
    wvt = np.concatenate([Wv, bv[None]]).astype(np.float16)
    ones = np.ones((1, H, W), np.float16)
    in_maps = []
    for b in range(B):
        x65 = np.concatenate([np.asarray(x[b]).astype(np.float16), ones])
        in_maps.append({
            "x65": np.ascontiguousarray(x65),
            "wqt": wqt, "wkt": wkt, "wvt": wvt,
            "mask": mask,
        })
    return in_maps


def run(inputs, trace=False, **spmd_kwargs):
    """Returns (output [8, 58, 58, 64] f32, BassKernelResults)."""
    from concourse import bass_utils

    nc, mask = _get_module()
    in_maps = _make_in_maps(
        inputs["x"], inputs["Wq"], inputs["bq"], inputs["Wk"], inputs["bk"],
        inputs["Wv"], inputs["bv"], mask,
    )
    res = bass_utils.run_bass_kernel_spmd(
        nc, in_maps, core_ids=list(range(B)), trace=trace, **spmd_kwargs,
    )
    out = np.stack(
        [res.results[b]["out"].reshape(HC, WC, C) for b in range(B)]
    ).astype(np.float32)
    return out, res


def kernel(**inputs) -> np.ndarray:
    return run(inputs)[0]


# revision 8
# speedup vs baseline: 1.0906x; 1.0249x over previous
"""Trainium2 Bass kernel for ConvolutionalSelfAttention.

Problem (hardcoded): x [8, 64, 64, 64] f32 (B, C, H, W), Wq/Wk/Wv [64, 64],
bq/bk/bv [64]. 7x7 'valid' windows (58x58 per image), query = window center
pixel, keys/values = whole window, softmax(q.k/sqrt(C)) @ v.
Output [8, 58, 58, 64] f32.

Strategy: data-parallel over batch, one image per NeuronCore (8 cores).
Per core:
  - inputs shipped fp16 (x + ones row, weights + bias rows)
  - QKV projected once per PIXEL: q, k channel-major [C=64p, H, W];
    v spatial-major [32x128p, 65] (65th col = ones -> softmax denominator)
  - windowed attention over window-row pairs (i, i+1), padded to 128
    columns (full image rows as rhs) so every stationary operand has
    exactly 128 weight columns -> fast weight load. Column col of a pair
    maps to window (i + col//64, col%64 - 3); the 12 out-of-range columns
    per pair are masked to a single arbitrary key so their denominators
    stay finite, and never stored.
"""

import numpy as np

B, C, H, W, K = 8, 64, 64, 64, 7
HC = WC = H - K + 1          # 58
N = HC * WC                  # 3364
NPAIR = HC // 2              # 29 window-row pairs
JW = 2 * WC                  # 116 windows per pair
SCALE = float(1.0 / np.sqrt(C))
SCALAR_DMA = True           # issue half the DMAs from the ACT HWDGE ring

_CACHE = {}


def _build_mask_np():
    """[128, 4, 128] multiplicative band mask in the padded-column layout."""
    kk = np.arange(128)[:, None, None]
    c = np.arange(4)[None, :, None]
    col = np.arange(128)[None, None, :]
    k_local = c * 128 + kk            # [0, 512) position within the row band
    dI, jp = k_local // W, k_local % W
    jb, j = col // 64, col % 64 - 3
    ok = (j >= 0) & (j < WC) & (dI - jb >= 0) & (dI - jb < K) \
        & (jp - j >= 0) & (jp - j < K)
    m = ok.astype(np.float16)
    # pad columns (j out of range): one arbitrary key so denom stays finite
    m[0, 0, (np.arange(128) % 64 - 3 < 0) | (np.arange(128) % 64 - 3 >= WC)] = 1.0
    return m


def _build_module():
    import concourse.tile as tile
    from concourse import bacc, mybir

    dt = mybir.dt
    f32 = dt.float32
    f16 = dt.float16

    nc = bacc.Bacc(
        "TRN2", target_bir_lowering=False, debug=False, enable_asserts=False,
        num_devices=8,
    )

    x_d = nc.dram_tensor("x65", [65, H, W], f16, kind="ExternalInput").ap()
    wq_d = nc.dram_tensor("wqt", [65, C], f16, kind="ExternalInput").ap()
    wk_d = nc.dram_tensor("wkt", [65, C], f16, kind="ExternalInput").ap()
    wv_d = nc.dram_tensor("wvt", [65, C], f16, kind="ExternalInput").ap()
    mask_d = nc.dram_tensor("mask", [128, 4, 128], f16, kind="ExternalInput").ap()
    out_d = nc.dram_tensor("out", [N, C], f32, kind="ExternalOutput").ap()

    with tile.TileContext(nc) as tc:
        with (
            tc.tile_pool(name="const", bufs=1) as const,
            tc.tile_pool(name="qk", bufs=1) as qkpool,
            tc.tile_pool(name="attn", bufs=4) as attnpool,
            tc.tile_pool(name="fin", bufs=4) as finpool,
        ):
            x_sb = const.tile([65, H, W], f16)
            wq_sb = const.tile([65, C], f16)
            wk_sb = const.tile([65, C], f16)
            wv_sb = const.tile([65, C], f16)
            mask_sb = const.tile([128, 4, 128], f16)
            q_sb = qkpool.tile([C, H, W], f16, tag="q")
            k_sb = qkpool.tile([C, H, W], f16, tag="k")
            v_sb = qkpool.tile([128, 32, C + 1], f16, tag="v")

            eng2 = nc.scalar if SCALAR_DMA else nc.sync
            eng2.dma_start(wq_sb[:], wq_d[:])
            eng2.dma_start(wk_sb[:], wk_d[:])
            eng2.dma_start(wv_sb[:], wv_d[:])
            nc.sync.dma_start(x_sb[:, 0:32, :], x_d[:, 0:32, :])
            eng2.dma_start(x_sb[:, 32:64, :], x_d[:, 32:64, :])
            nc.sync.dma_start(mask_sb[:], mask_d[:])
            nc.gpsimd.memset(v_sb[:], 1.0)  # ones column (col 64 of each chunk)

            # ---- QKV projections ----
            with (
                tc.tile_pool(name="psqk", bufs=2, space="PSUM") as psqk,
                tc.tile_pool(name="psv", bufs=2, space="PSUM") as psv,
            ):
                # q, k channel-major: out[c_out, pix] = sum_cin W[cin, cout] x[cin, pix]
                # 2 matmuls (N=512 each) per 2-bank PSUM tile, 1 merged copy.
                for g in range(4):
                    ps = psqk.tile([C, 16, W], f32, tag="ps")
                    for h in range(2):
                        s = 2 * g + h
                        nc.tensor.matmul(
                            ps[:, 8 * h:8 * h + 8, :],
                            wq_sb[:],
                            x_sb[:, s * 8:(s + 1) * 8, :],
                        )
                    eng = nc.scalar.copy if g % 2 == 0 else nc.vector.tensor_copy
                    eng(q_sb[:, g * 16:(g + 1) * 16, :], ps[:])
                for g in range(4):
                    ps = psqk.tile([C, 16, W], f32, tag="ps")
                    for h in range(2):
                        s = 2 * g + h
                        nc.tensor.matmul(
                            ps[:, 8 * h:8 * h + 8, :],
                            wk_sb[:],
                            x_sb[:, s * 8:(s + 1) * 8, :],
                        )
                    eng = nc.scalar.copy if g % 2 == 1 else nc.vector.tensor_copy
                    eng(k_sb[:, g * 16:(g + 1) * 16, :], ps[:])
                # v spatial-major: out[pix, c_out] = sum_cin x[cin, pix] W[cin, cout]
                for g in range(8):
                    ps = psv.tile([128, 4, C], f32, tag="psv")
                    for h in range(4):
                        r = 4 * g + h
                        nc.tensor.matmul(
                            ps[:, h, :],
                            x_sb[:, 2 * r:2 * r + 2, :],
                            wv_sb[:],
                        )
                    eng = nc.scalar.copy if g % 2 == 0 else nc.vector.tensor_copy
                    eng(v_sb[:, 4 * g:4 * g + 4, 0:C], ps[:])

            # ---- banded attention over window-row pairs (padded-128 layout) ----
            with (
                tc.tile_pool(name="pssc", bufs=4, space="PSUM") as pssc,
                tc.tile_pool(name="psout", bufs=3, space="PSUM") as psout,
            ):
                scores = [None] * NPAIR

                def emit_scores(p):
                    i = 2 * p
                    sc = pssc.tile([128, 4, 128], f32, tag="sc")
                    q2 = q_sb[:, i + 3:i + 5, :]            # [64, 2, 64] = N=128
                    for c in range(4):
                        nc.tensor.matmul(
                            sc[:, c, :],
                            k_sb[:, i + 2 * c:i + 2 * c + 2, :],  # [64, 2, 64]
                            q2,
                        )
                    scores[p] = sc

                def emit_tail(p):
                    sc = scores[p]
                    ex = attnpool.tile([128, 4, 128], f16, tag="ex")
                    nc.scalar.activation(
                        ex[:], sc[:], mybir.ActivationFunctionType.Exp,
                        scale=SCALE,
                    )
                    at = attnpool.tile([128, 4, 128], f16, tag="at")
                    nc.vector.tensor_mul(at[:], ex[:], mask_sb[:])
                    ops = psout.tile([128, C + 1], f32, tag="ops")
                    for c in range(4):
                        nc.tensor.matmul(
                            ops[:],
                            at[:, c, :],            # [128, 128] -> FWL
                            v_sb[:, p + c, :],      # [128, 65]
                            start=(c == 0), stop=(c == 3),
                        )
                    recip = finpool.tile([128, 1], f32, tag="recip")
                    nc.vector.reciprocal(recip[:], ops[:, C:C + 1])
                    outt = finpool.tile([128, C], f32, tag="outt")
                    if p % 2 == 0:
                        nc.scalar.mul(outt[:], ops[:, 0:C], recip[:])
                    else:
                        nc.vector.tensor_scalar(
                            outt[:], ops[:, 0:C], recip[:], None,
                            mybir.AluOpType.mult,
                        )
                    # rows jb*64 + 3..61 hold windows (2p+jb, 0..58)
                    for jb in range(2):
                        eng = nc.sync if (p + jb) % 2 == 0 else eng2
                        eng.dma_start(
                            out_d[p * JW + jb * WC: p * JW + (jb + 1) * WC, :],
                            outt[jb * 64 + 3: jb * 64 + 3 + WC, :],
                        )

                LAG = 3
                for p in range(NPAIR):
                    emit_scores(p)
                    if p >= LAG:
                        emit_tail(p - LAG)
                for p in range(NPAIR - LAG, NPAIR):
                    emit_tail(p)

    nc.compile()
    return nc


def _get_module():
    if "nc" not in _CACHE:
        _CACHE["nc"] = _build_module()
        _CACHE["mask"] = _build_mask_np()
    return _CACHE["nc"], _CACHE["mask"]


def _make_in_maps(x, Wq, bq, Wk, bk, Wv, bv, mask):
    wqt = np.concatenate([Wq, bq[None]]).astype(np.float16)
    wkt = np.concatenate([Wk, bk[None]]).astype(np.float16)
    wvt = np.concatenate([Wv, bv[None]]).astype(np.float16)
    ones = np.ones((1, H, W), np.float16)
    in_maps = []
    for b in range(B):
        x65 = np.concatenate([np.asarray(x[b]).astype(np.float16), ones])
        in_maps.append({
            "x65": np.ascontiguousarray(x65),
            "wqt": wqt, "wkt": wkt, "wvt": wvt,
            "mask": mask,
        })
    return in_maps


def run(inputs, trace=False, **spmd_kwargs):
    """Returns (output [8, 58, 58, 64] f32, BassKernelResults)."""
    from concourse import bass_utils

    nc, mask = _get_module()
    in_maps = _make_in_maps(
        inputs["x"], inputs["Wq"], inputs["bq"], inputs["Wk"], inputs["bk"],
        inputs["Wv"], inputs["bv"], mask,
    )
    res = bass_utils.run_bass_kernel_spmd(
        nc, in_maps, core_ids=list(range(B)), trace=trace, **spmd_kwargs,
    )
    out = np.stack(
        [res.results[b]["out"].reshape(HC, WC, C) for b in range(B)]
    ).astype(np.float32)
    return out, res


def kernel(**inputs) -> np.ndarray:
    return run(inputs)[0]


# revision 9
# speedup vs baseline: 1.2523x; 1.1483x over previous
"""Trainium2 Bass kernel for ConvolutionalSelfAttention.

Problem (hardcoded): x [8, 64, 64, 64] f32 (B, C, H, W), Wq/Wk/Wv [64, 64],
bq/bk/bv [64]. 7x7 'valid' windows (58x58 per image), query = window center
pixel, keys/values = whole window, softmax(q.k/sqrt(C)) @ v.
Output [8, 58, 58, 64] f32.

Strategy: data-parallel over batch, one image per NeuronCore (8 cores).
Per core:
  - inputs shipped fp16 (x + ones row, weights + bias rows)
  - QKV projected once per PIXEL: q, k channel-major [C=64p, H, W];
    v spatial-major [32x128p, 65] (65th col = ones -> softmax denominator)
  - windowed attention over window-row pairs (i, i+1), padded to 128
    columns (full image rows as rhs) so every stationary operand has
    exactly 128 weight columns -> fast weight load. Column col of a pair
    maps to window (i + col//64, col%64 - 3); the 12 out-of-range columns
    per pair are masked to a single arbitrary key so their denominators
    stay finite, and never stored.
"""

import numpy as np

B, C, H, W, K = 8, 64, 64, 64, 7
HC = WC = H - K + 1          # 58
N = HC * WC                  # 3364
NPAIR = HC // 2              # 29 window-row pairs
JW = 2 * WC                  # 116 windows per pair
SCALE = float(1.0 / np.sqrt(C))
SCALAR_DMA = True           # issue half the DMAs from the ACT HWDGE ring

_CACHE = {}


def _build_mask_np():
    """[128, 4, 128] multiplicative band mask in the padded-column layout."""
    kk = np.arange(128)[:, None, None]
    c = np.arange(4)[None, :, None]
    col = np.arange(128)[None, None, :]
    k_local = c * 128 + kk            # [0, 512) position within the row band
    dI, jp = k_local // W, k_local % W
    jb, j = col // 64, col % 64 - 3
    ok = (j >= 0) & (j < WC) & (dI - jb >= 0) & (dI - jb < K) \
        & (jp - j >= 0) & (jp - j < K)
    m = ok.astype(np.float16)
    # pad columns (j out of range): one arbitrary key so denom stays finite
    m[0, 0, (np.arange(128) % 64 - 3 < 0) | (np.arange(128) % 64 - 3 >= WC)] = 1.0
    return m


def _build_module():
    import concourse.tile as tile
    from concourse import bacc, mybir

    dt = mybir.dt
    f32 = dt.float32
    f16 = dt.float16

    nc = bacc.Bacc(
        "TRN2", target_bir_lowering=False, debug=False, enable_asserts=False,
        num_devices=8,
    )

    x_d = nc.dram_tensor("x65", [65, H, W], f16, kind="ExternalInput").ap()
    wq_d = nc.dram_tensor("wqt", [65, C], f16, kind="ExternalInput").ap()
    wk_d = nc.dram_tensor("wkt", [65, C], f16, kind="ExternalInput").ap()
    wv_d = nc.dram_tensor("wvt", [65, C], f16, kind="ExternalInput").ap()
    mask_d = nc.dram_tensor("mask", [128, 4, 128], f16, kind="ExternalInput").ap()
    out_d = nc.dram_tensor("out", [N, C], f32, kind="ExternalOutput").ap()

    with tile.TileContext(nc) as tc:
        with (
            tc.tile_pool(name="const", bufs=1) as const,
            tc.tile_pool(name="qk", bufs=1) as qkpool,
            tc.tile_pool(name="attn", bufs=4) as attnpool,
            tc.tile_pool(name="fin", bufs=4) as finpool,
        ):
            x_sb = const.tile([65, H, W], f16)
            wq_sb = const.tile([65, C], f16)
            wk_sb = const.tile([65, C], f16)
            wv_sb = const.tile([65, C], f16)
            mask_sb = const.tile([128, 4, 128], f16)
            q_sb = qkpool.tile([C, H, W], f16, tag="q")
            k_sb = qkpool.tile([C, H, W], f16, tag="k")
            v_sb = qkpool.tile([128, 32, C + 1], f16, tag="v")

            eng2 = nc.scalar if SCALAR_DMA else nc.sync
            eng2.dma_start(wq_sb[:], wq_d[:])
            eng2.dma_start(wk_sb[:], wk_d[:])
            eng2.dma_start(wv_sb[:], wv_d[:])
            nc.sync.dma_start(x_sb[:, 0:32, :], x_d[:, 0:32, :])
            eng2.dma_start(x_sb[:, 32:64, :], x_d[:, 32:64, :])
            nc.sync.dma_start(mask_sb[:], mask_d[:])
            nc.gpsimd.memset(v_sb[:], 1.0)  # ones column (col 64 of each chunk)

            # ---- QKV projections ----
            with (
                tc.tile_pool(name="psqk", bufs=2, space="PSUM") as psqk,
                tc.tile_pool(name="psv", bufs=2, space="PSUM") as psv,
            ):
                # q, k channel-major: out[c_out, pix] = sum_cin W[cin, cout] x[cin, pix]
                # 2 matmuls (N=512 each) per 2-bank PSUM tile, 1 merged copy.
                for g in range(4):
                    ps = psqk.tile([C, 16, W], f32, tag="ps")
                    for h in range(2):
                        s = 2 * g + h
                        nc.tensor.matmul(
                            ps[:, 8 * h:8 * h + 8, :],
                            wq_sb[:],
                            x_sb[:, s * 8:(s + 1) * 8, :],
                        )
                    eng = nc.scalar.copy if g % 2 == 0 else nc.vector.tensor_copy
                    eng(q_sb[:, g * 16:(g + 1) * 16, :], ps[:])
                for g in range(4):
                    ps = psqk.tile([C, 16, W], f32, tag="ps")
                    for h in range(2):
                        s = 2 * g + h
                        nc.tensor.matmul(
                            ps[:, 8 * h:8 * h + 8, :],
                            wk_sb[:],
                            x_sb[:, s * 8:(s + 1) * 8, :],
                        )
                    eng = nc.scalar.copy if g % 2 == 1 else nc.vector.tensor_copy
                    eng(k_sb[:, g * 16:(g + 1) * 16, :], ps[:])
                # v spatial-major: out[pix, c_out] = sum_cin x[cin, pix] W[cin, cout]
                for g in range(8):
                    ps = psv.tile([128, 4, C], f32, tag="psv")
                    for h in range(4):
                        r = 4 * g + h
                        nc.tensor.matmul(
                            ps[:, h, :],
                            x_sb[:, 2 * r:2 * r + 2, :],
                            wv_sb[:],
                        )
                    eng = nc.scalar.copy if g % 2 == 0 else nc.vector.tensor_copy
                    eng(v_sb[:, 4 * g:4 * g + 4, 0:C], ps[:])

            # ---- banded attention over window-row pairs (padded-128 layout) ----
            with (
                tc.tile_pool(name="pssc", bufs=4, space="PSUM") as pssc,
                tc.tile_pool(name="psout", bufs=3, space="PSUM") as psout,
            ):
                scores = [None] * NPAIR

                def emit_scores(p):
                    i = 2 * p
                    sc = pssc.tile([128, 4, 128], f32, tag="sc")
                    q2 = q_sb[:, i + 3:i + 5, :]            # [64, 2, 64] = N=128
                    for c in range(4):
                        nc.tensor.matmul(
                            sc[:, c, :],
                            k_sb[:, i + 2 * c:i + 2 * c + 2, :],  # [64, 2, 64]
                            q2,
                        )
                    scores[p] = sc

                def emit_tail(p):
                    sc = scores[p]
                    ex = attnpool.tile([128, 4, 128], f16, tag="ex")
                    nc.scalar.activation(
                        ex[:], sc[:], mybir.ActivationFunctionType.Exp,
                        scale=SCALE,
                    )
                    at = attnpool.tile([128, 4, 128], f16, tag="at")
                    nc.vector.tensor_mul(at[:], ex[:], mask_sb[:])
                    ops = psout.tile([128, C + 1], f32, tag="ops")
                    for c in range(4):
                        nc.tensor.matmul(
                            ops[:],
                            at[:, c, :],            # [128, 128] -> FWL
                            v_sb[:, p + c, :],      # [128, 65]
                            start=(c == 0), stop=(c == 3),
                        )
                    recip = finpool.tile([128, 1], f32, tag="recip")
                    nc.vector.reciprocal(recip[:], ops[:, C:C + 1])
                    outt = finpool.tile([128, C], f32, tag="outt")
                    if p % 2 == 0:
                        nc.scalar.mul(outt[:], ops[:, 0:C], recip[:])
                    else:
                        nc.vector.tensor_scalar(
                            outt[:], ops[:, 0:C], recip[:], None,
                            mybir.AluOpType.mult,
                        )
                    # rows jb*64 + 3..61 hold windows (2p+jb, 0..58)
                    for jb in range(2):
                        eng = nc.sync if p % 2 == 0 else nc.gpsimd
                        eng.dma_start(
                            out_d[p * JW + jb * WC: p * JW + (jb + 1) * WC, :],
                            outt[jb * 64 + 3: jb * 64 + 3 + WC, :],
                        )

                LAG = 3
                for p in range(NPAIR):
                    emit_scores(p)
                    if p >= LAG:
                        emit_tail(p - LAG)
                for p in range(NPAIR - LAG, NPAIR):
                    emit_tail(p)

    nc.compile()
    return nc


def _get_module():
    if "nc" not in _CACHE:
        _CACHE["nc"] = _build_module()
        _CACHE["mask"] = _build_mask_np()
    return _CACHE["nc"], _CACHE["mask"]


def _make_in_maps(x, Wq, bq, Wk, bk, Wv, bv, mask):
    wqt = np.concatenate([Wq, bq[None]]).astype(np.float16)
    wkt = np.concatenate([Wk, bk[None]]).astype(np.float16)
    wvt = np.concatenate([Wv, bv[None]]).astype(np.float16)
    ones = np.ones((1, H, W), np.float16)
    in_maps = []
    for b in range(B):
        x65 = np.concatenate([np.asarray(x[b]).astype(np.float16), ones])
        in_maps.append({
            "x65": np.ascontiguousarray(x65),
            "wqt": wqt, "wkt": wkt, "wvt": wvt,
            "mask": mask,
        })
    return in_maps


def run(inputs, trace=False, **spmd_kwargs):
    """Returns (output [8, 58, 58, 64] f32, BassKernelResults)."""
    from concourse import bass_utils

    nc, mask = _get_module()
    in_maps = _make_in_maps(
        inputs["x"], inputs["Wq"], inputs["bq"], inputs["Wk"], inputs["bk"],
        inputs["Wv"], inputs["bv"], mask,
    )
    res = bass_utils.run_bass_kernel_spmd(
        nc, in_maps, core_ids=list(range(B)), trace=trace, **spmd_kwargs,
    )
    out = np.stack(
        [res.results[b]["out"].reshape(HC, WC, C) for b in range(B)]
    ).astype(np.float32)
    return out, res


def kernel(**inputs) -> np.ndarray:
    return run(inputs)[0]
